# revision 13
# baseline (speedup 1.0000x reference)
"""GNN edge-softmax attention kernel for 8 TRN2 NeuronCores.

Problem: nn_Attention_3015067042351 (gnn_message_passing).
  N=50000 nodes, E=1600000 edges, C=128, H=8.
  alpha = segment_softmax(leaky_relu((x[row]@W1 + x[col]@W2 + b)*|ea|)*100, by=row)

Strategy:
  - Algebra: x[row]@W1 + x[col]@W2 = u[row] + v[col] with u = x@W1+b, v = x@W2
    (per-edge gather shrinks from 512B to 32B).
  - Host "sharding": partition nodes (softmax segments) across 8 devices by
    edge count (snake order) so per-device load balances; sort segments by
    count so tiles of 128 segments share a small per-tile K (max edges/seg);
    lay edges out dense [segment, K] so the softmax is tile-local (no
    cross-device reduction, no scatter).
  - Device: matmul u,v for the local node shard; AllGather v; per tile of
    128 segments gather v[col] rows via indirect DMA (accumulated onto a
    broadcast u prefill), then mul/leaky/max/exp/sum/normalize; write out.
  - Host unpermutes dense output back to original edge order.

Self-contained: hardcodes shapes; only needs the /opt/trn_rl_repo runtime.
"""

import sys

if "/opt/trn_rl_repo" not in sys.path:
    sys.path.insert(0, "/opt/trn_rl_repo")

import numpy as np

N, E, C, H = 50000, 1600000, 128, 8
NDEV = 8
P = 128
NTILES = 49            # tiles of 128 segments per device
SEG_PER_DEV = NTILES * P   # 6272
NPAD = NDEV * SEG_PER_DEV  # 50176
SENTINEL_LOCAL = SEG_PER_DEV - 1  # per-device v row holding -1e33 (masks pads)
GROUP = 1              # tiles per DMA/gather group
NGROUPS = NTILES // GROUP
SENTINEL_VAL = -1.0e33

_COMPILED = {}  # (Ksched tuple) -> (nc, names)


# --------------------------------------------------------------------------
# Host-side plan
# --------------------------------------------------------------------------

def _build_plan(edge_index, edge_attr):
    row = np.asarray(edge_index[0])
    col = np.asarray(edge_index[1])
    ea = np.asarray(edge_attr, dtype=np.float32)
    M = E + N
    row_full = np.concatenate([row, np.arange(N, dtype=row.dtype)])
    col_full = np.concatenate([col, np.arange(N, dtype=col.dtype)])
    aea_full = np.concatenate([np.abs(ea), np.ones(N, np.float32)]).astype(np.float32)

    counts = np.bincount(row_full, minlength=N)
    counts_pad = np.concatenate([counts, np.zeros(NPAD - N, np.int64)])
    order_nodes = np.argsort(-counts_pad, kind="stable")
    # snake-assign sorted nodes to (device, local_pos) to balance edge counts
    g = np.arange(NPAD) // NDEV
    r = np.arange(NPAD) % NDEV
    dev_of_sorted = np.where(g % 2 == 0, r, NDEV - 1 - r).astype(np.int32)
    loc_of_sorted = g.astype(np.int32)
    node_dev = np.empty(NPAD, np.int32)
    node_loc = np.empty(NPAD, np.int32)
    node_dev[order_nodes] = dev_of_sorted
    node_loc[order_nodes] = loc_of_sorted
    node_glb = node_dev.astype(np.int64) * SEG_PER_DEV + node_loc

    # shared per-tile K schedule (max count in tile across all devices)
    counts_sorted = counts_pad[order_nodes]
    tile_of_sorted = loc_of_sorted // P
    Ksched = np.zeros(NTILES, np.int64)
    np.maximum.at(Ksched, tile_of_sorted, counts_sorted)
    Ksched = np.maximum(Ksched, 2)
    Ksched = ((Ksched + 1) // 2) * 2

    # group layout: per group gi, tiles j = gi*GROUP..+GROUP
    Kg = Ksched.reshape(NGROUPS, GROUP)
    SK = Kg.sum(axis=1)                      # slots per partition-row per group
    off_in_group = np.concatenate(
        [np.zeros((NGROUPS, 1), np.int64), np.cumsum(Kg, axis=1)[:, :-1]], axis=1
    )
    group_base = np.concatenate([[0], np.cumsum(P * SK)])  # slot base per group
    TOT = int(group_base[-1])

    # per-edge placement
    e_dev = node_dev[row_full]
    e_loc = node_loc[row_full]
    sort_key = e_dev.astype(np.int64) * NPAD + e_loc
    order_e = np.argsort(sort_key, kind="stable")
    sk = sort_key[order_e]
    starts = np.flatnonzero(np.concatenate([[True], sk[1:] != sk[:-1]]))
    run_id = np.cumsum(np.concatenate([[False], sk[1:] != sk[:-1]]))
    kpos = np.empty(M, np.int64)
    kpos[order_e] = np.arange(M) - starts[run_id]

    tile_j = (e_loc // P).astype(np.int64)
    part_p = (e_loc % P).astype(np.int64)
    gi = tile_j // GROUP
    jj = tile_j % GROUP
    pos = group_base[gi] + part_p * SK[gi] + off_in_group[gi, jj] + kpos

    colg = np.empty((NDEV, TOT), np.int32)
    colg[:] = (np.arange(NDEV, dtype=np.int32) * SEG_PER_DEV + SENTINEL_LOCAL)[:, None]
    aead = np.ones((NDEV, TOT), np.float32)
    colg[e_dev, pos] = node_glb[col_full].astype(np.int32)
    aead[e_dev, pos] = aea_full

    # meta arrays: per group block [P, 2*SK]: [ci row (SK) | ae row (SK)]
    meta = np.empty((NDEV, 2 * TOT), np.int32)
    for gidx in range(NGROUPS):
        b0 = int(group_base[gidx])
        skg = int(SK[gidx])
        blk_ci = colg[:, b0:b0 + P * skg].reshape(NDEV, P, skg)
        blk_ae = aead[:, b0:b0 + P * skg].reshape(NDEV, P, skg).view(np.int32)
        m2 = np.concatenate([blk_ci, blk_ae], axis=2)  # [NDEV, P, 2*SK]
        meta[:, 2 * b0:2 * (b0 + P * skg)] = m2.reshape(NDEV, -1)

    return dict(
        row_full=row_full, col_full=col_full,
        node_glb=node_glb, Ksched=Ksched, SK=SK,
        off_in_group=off_in_group, group_base=group_base, TOT=TOT,
        e_dev=e_dev, pos=pos, meta=meta,
    )


# --------------------------------------------------------------------------
# Device kernel
# --------------------------------------------------------------------------

def _build_bass(Ksched, SK, off_in_group, group_base, TOT, debug_dump=None):
    import concourse.bass as bass
    import concourse.mybir as mybir
    import concourse.tile as tile
    from concourse import bacc
    from concourse.bass import IndirectOffsetOnAxis
    from concourse.ap import AP

    f32 = mybir.dt.float32
    i32 = mybir.dt.int32
    Alu = mybir.AluOpType
    Act = mybir.ActivationFunctionType
    X = mybir.AxisListType.X

    nc = bacc.Bacc("TRN2", target_bir_lowering=False, debug=False,
                   num_devices=NDEV, dynamic_dma_scratch_size=1 << 16)

    xt = nc.dram_tensor("xt", [P, SEG_PER_DEV], f32, kind="ExternalInput")
    wb = nc.dram_tensor("wb", [P, 2 * H], f32, kind="ExternalInput")
    b2 = nc.dram_tensor("b2", [1, 2 * H], f32, kind="ExternalInput")
    meta = nc.dram_tensor("meta", [2 * TOT], i32, kind="ExternalInput")
    outd = nc.dram_tensor("out", [TOT, H], f32, kind="ExternalOutput")

    def bc(ap, ins_at, n):
        """Insert a broadcast (step-0) dim of size n at position ins_at."""
        dims = list(ap.ap)
        dims.insert(ins_at, [0, n])
        return AP(ap.tensor, ap.offset, dims)

    with tile.TileContext(nc) as tc:
        with (
            tc.tile_pool(name="const", bufs=1) as cpool,
            tc.tile_pool(name="work", bufs=2) as wpool,
            tc.tile_pool(name="small", bufs=3) as spool,
            tc.tile_pool(name="psum", bufs=2, space="PSUM") as ppool,
            tc.tile_pool(name="dram", bufs=1, space="DRAM") as dpool,
        ):
            # ---------- phase A: u, v = x @ [W1|W2] + [b|0] ----------
            xt_sb = cpool.tile([P, SEG_PER_DEV], f32)
            nc.sync.dma_start(out=xt_sb[:, :], in_=xt.ap())
            wb_sb = cpool.tile([P, 2 * H], f32)
            nc.sync.dma_start(out=wb_sb[:, :], in_=wb.ap())
            b2_sb = cpool.tile([1, 2 * H], f32)
            nc.sync.dma_start(out=b2_sb[:, :], in_=b2.ap())
            ones_sb = cpool.tile([1, P], f32)
            nc.vector.memset(ones_sb[:, :], 1.0)

            u_sb = cpool.tile([P, NTILES * H], f32)
            v_st = cpool.tile([P, NTILES * H], f32)
            for j in range(NTILES):
                ps = ppool.tile([P, 2 * H], f32, tag="ps")
                nc.tensor.matmul(ps[:, :], lhsT=xt_sb[:, j * P:(j + 1) * P],
                                 rhs=wb_sb[:, :], start=True, stop=False)
                nc.tensor.matmul(ps[:, :], lhsT=ones_sb[:, :], rhs=b2_sb[:, :],
                                 start=False, stop=True)
                nc.scalar.copy(u_sb[:, j * H:(j + 1) * H], ps[:, 0:H])
                nc.vector.tensor_copy(v_st[:, j * H:(j + 1) * H], ps[:, H:2 * H])
            # (sentinel row: host plants an x row with x@W2 = SENTINEL_VAL)
            v_shard = dpool.tile([SEG_PER_DEV, H], f32)
            v_all = dpool.tile([NPAD, H], f32, addr_space="Shared")
            nc.sync.dma_start(
                out=v_shard[:, :].rearrange("(j p) h -> p j h", p=P),
                in_=v_st[:, :].rearrange("p (j h) -> p j h", h=H),
            )
            nc.gpsimd.collective_compute(
                "AllGather", mybir.AluOpType.bypass,
                replica_groups=[list(range(NDEV))],
                ins=[v_shard[:, :].opt()],
                outs=[v_all[:, :].opt()],
            )

            # ---------- phase B: per-group gather + per-tile softmax ----------
            out_flat = outd.ap().rearrange("t h -> (t h)")
            meta_flat = meta.ap()
            for gi in range(NGROUPS):
                skg = int(SK[gi])
                gb = int(group_base[gi])
                mg = wpool.tile([P, 2 * skg], i32, tag="mg")
                nc.sync.dma_start(
                    out=mg[:, :],
                    in_=meta_flat[2 * gb:2 * (gb + P * skg)].rearrange(
                        "(p x) -> p x", x=2 * skg),
                )
                gg = wpool.tile([P, skg * H], f32, tag="gg")
                # prefill with u[segment] broadcast over k
                for jj in range(GROUP):
                    j = gi * GROUP + jj
                    K = int(Ksched[j])
                    off = int(off_in_group[gi][jj])
                    ub = u_sb[:, j * H:(j + 1) * H]
                    nc.scalar.copy(
                        gg[:, off * H:(off + K) * H].rearrange(
                            "p (k h) -> p k h", h=H),
                        bc(ub, 1, K),
                    )
                # gather v rows: gg += v_all[ci]  (HW indirect DMA consumes one
                # index per partition per instruction -> one call per k column)
                for kk in range(skg):
                    nc.gpsimd.indirect_dma_start(
                        out=gg[:, kk * H:(kk + 1) * H],
                        out_offset=None,
                        in_=v_all[:, :],
                        in_offset=IndirectOffsetOnAxis(ap=mg[:, kk:kk + 1], axis=0),
                        compute_op=mybir.AluOpType.add,
                    )
                # t = g * |ea| ; la = leaky(t) = max(t, 0.2t)
                ae_ap = mg[:, skg:2 * skg].bitcast(f32)
                tg = wpool.tile([P, skg * H], f32, tag="tg")
                nc.vector.tensor_tensor(
                    out=tg[:, :].rearrange("p (k h) -> p k h", h=H),
                    in0=gg[:, :].rearrange("p (k h) -> p k h", h=H),
                    in1=bc(ae_ap, 2, H),
                    op=Alu.mult,
                )
                la = wpool.tile([P, skg * H], f32, tag="la")
                nc.vector.scalar_tensor_tensor(
                    out=la[:, :], in0=tg[:, :], scalar=0.2, in1=tg[:, :],
                    op0=Alu.mult, op1=Alu.max,
                )
                mx = spool.tile([P, GROUP * H], f32, tag="mx")
                sm = spool.tile([P, GROUP * H], f32, tag="sm")
                rc = spool.tile([P, GROUP * H], f32, tag="rc")
                eg = wpool.tile([P, skg * H], f32, tag="eg")
                og = wpool.tile([P, skg * H], f32, tag="og")
                for jj in range(GROUP):
                    j = gi * GROUP + jj
                    K = int(Ksched[j])
                    off = int(off_in_group[gi][jj])
                    sl = slice(off * H, (off + K) * H)
                    la_t = la[:, sl]
                    mx_t = mx[:, jj * H:(jj + 1) * H]
                    nc.vector.tensor_reduce(
                        out=mx_t, in_=la_t.rearrange("p (k h) -> p h k", h=H),
                        axis=X, op=Alu.max,
                    )
                    # d = la - m  (into eg slice temp), e = exp(100 d)
                    nc.vector.tensor_tensor(
                        out=eg[:, sl].rearrange("p (k h) -> p k h", h=H),
                        in0=la_t.rearrange("p (k h) -> p k h", h=H),
                        in1=bc(mx_t, 1, K),
                        op=Alu.subtract,
                    )
                    nc.scalar.activation(
                        out=eg[:, sl], in_=eg[:, sl], func=Act.Exp, scale=100.0,
                    )
                    nc.vector.tensor_reduce(
                        out=sm[:, jj * H:(jj + 1) * H],
                        in_=eg[:, sl].rearrange("p (k h) -> p h k", h=H),
                        axis=X, op=Alu.add,
                    )
                nc.vector.reciprocal(rc[:, :], sm[:, :])
                for jj in range(GROUP):
                    j = gi * GROUP + jj
                    K = int(Ksched[j])
                    off = int(off_in_group[gi][jj])
                    sl = slice(off * H, (off + K) * H)
                    nc.vector.tensor_tensor(
                        out=og[:, sl].rearrange("p (k h) -> p k h", h=H),
                        in0=eg[:, sl].rearrange("p (k h) -> p k h", h=H),
                        in1=bc(rc[:, jj * H:(jj + 1) * H], 1, K),
                        op=Alu.mult,
                    )
                dump = {"g": gg, "t": tg, "la": la, "e": eg, None: og}[debug_dump]
                nc.sync.dma_start(
                    out=out_flat[gb * H:(gb + P * skg) * H].rearrange(
                        "(p x) -> p x", x=skg * H),
                    in_=dump[:, :],
                )
    nc.compile()
    return nc


# --------------------------------------------------------------------------
# Entry point
# --------------------------------------------------------------------------

def _prepare(inputs):
    plan = _build_plan(inputs["edge_index"], inputs["edge_attr"])
    x = np.asarray(inputs["x"], np.float32)
    W = np.asarray(inputs["W"], np.float32)
    b = np.asarray(inputs["b"], np.float32)

    x_glb = np.zeros((NPAD, C), np.float32)
    x_glb[plan["node_glb"][:N]] = x
    # sentinel: craft an x row so x @ W2 == SENTINEL_VAL on every head
    # (min-norm solve; the sentinel position is a dummy node on each device)
    W2 = W[C:].astype(np.float64)
    xs = np.linalg.lstsq(W2.T, np.full(H, SENTINEL_VAL), rcond=None)[0]
    for d in range(NDEV):
        x_glb[d * SEG_PER_DEV + SENTINEL_LOCAL] = xs.astype(np.float32)
    xt_full = np.ascontiguousarray(x_glb.T)  # [C, NPAD]
    wbm = np.ascontiguousarray(np.concatenate([W[:C], W[C:]], axis=1))  # [128,16]
    b2 = np.concatenate([b, np.zeros(H, np.float32)])[None, :].astype(np.float32)

    in_maps = []
    for d in range(NDEV):
        in_maps.append({
            "xt": np.ascontiguousarray(
                xt_full[:, d * SEG_PER_DEV:(d + 1) * SEG_PER_DEV]),
            "wb": wbm,
            "b2": b2,
            "meta": np.ascontiguousarray(plan["meta"][d]),
        })
    return plan, in_maps


def _get_nc(plan, debug_dump=None):
    key = (tuple(plan["Ksched"].tolist()), debug_dump)
    if key not in _COMPILED:
        _COMPILED[key] = _build_bass(
            plan["Ksched"], plan["SK"], plan["off_in_group"],
            plan["group_base"], plan["TOT"], debug_dump=debug_dump)
    return _COMPILED[key]


def run(inputs, trace=False, debug_dump=None):
    """Returns ((alpha, rowcol), BassKernelResults)."""
    from concourse.bass_utils import run_bass_kernel_spmd

    plan, in_maps = _prepare(inputs)
    nc = _get_nc(plan, debug_dump)
    res = run_bass_kernel_spmd(nc, in_maps, core_ids=list(range(NDEV)),
                               trace=trace)
    outs = np.stack([r["out"] for r in res.results])  # [NDEV, TOT, H]
    alpha = outs[plan["e_dev"], plan["pos"]].astype(np.float32)
    rowcol = np.stack([plan["row_full"], plan["col_full"]]).astype(np.int32)
    return (alpha, rowcol), res


def kernel(**inputs):
    (alpha, rowcol), _ = run(inputs, trace=False)
    return alpha, rowcol


if __name__ == "__main__":
    from host_proto import np_setup_inputs, np_reference
    ins = np_setup_inputs()
    (alpha, rowcol), res = run(ins, trace=False)
    exp, rc = np_reference(**ins)
    err = np.linalg.norm(alpha - exp) / np.linalg.norm(exp)
    print("rel l2:", err)
    print("max abs diff:", np.abs(alpha - exp).max())
    print("rowcol ok:", np.array_equal(rowcol, rc))


# revision 14
# speedup vs baseline: 1.4450x; 1.4450x over previous
"""GNN edge-softmax attention kernel for 8 TRN2 NeuronCores.

Problem: nn_Attention_3015067042351 (gnn_message_passing).
  N=50000 nodes, E=1600000 edges, C=128, H=8.
  alpha = segment_softmax(leaky_relu((x[row]@W1 + x[col]@W2 + b)*|ea|)*100, by=row)

Strategy:
  - Algebra: x[row]@W1 + x[col]@W2 = u[row] + v[col] with u = x@W1+b, v = x@W2
    (per-edge gather shrinks from 512B to 32B).
  - Host "sharding": partition nodes (softmax segments) across 8 devices by
    edge count (snake order) so per-device load balances; sort segments by
    count so tiles of 128 segments share a small per-tile K (max edges/seg);
    lay edges out dense [segment, K] so the softmax is tile-local (no
    cross-device reduction, no scatter).
  - Device: matmul u,v for the local node shard; AllGather v; per tile of
    128 segments gather v[col] rows via indirect DMA (accumulated onto a
    broadcast u prefill), then mul/leaky/max/exp/sum/normalize; write out.
  - Host unpermutes dense output back to original edge order.

Self-contained: hardcodes shapes; only needs the /opt/trn_rl_repo runtime.
"""

import sys

if "/opt/trn_rl_repo" not in sys.path:
    sys.path.insert(0, "/opt/trn_rl_repo")

import numpy as np

N, E, C, H = 50000, 1600000, 128, 8
NDEV = 8
P = 128
NTILES = 49            # tiles of 128 segments per device
SEG_PER_DEV = NTILES * P   # 6272
NPAD = NDEV * SEG_PER_DEV  # 50176
SENTINEL_LOCAL = SEG_PER_DEV - 1  # per-device v row holding -1e33 (masks pads)
GROUP = 1              # tiles per DMA/gather group
NGROUPS = NTILES // GROUP
SENTINEL_VAL = -1.0e33

_COMPILED = {}  # (Ksched tuple) -> (nc, names)


# --------------------------------------------------------------------------
# Host-side plan
# --------------------------------------------------------------------------

def _build_plan(edge_index, edge_attr):
    row = np.asarray(edge_index[0])
    col = np.asarray(edge_index[1])
    ea = np.asarray(edge_attr, dtype=np.float32)
    M = E + N
    row_full = np.concatenate([row, np.arange(N, dtype=row.dtype)])
    col_full = np.concatenate([col, np.arange(N, dtype=col.dtype)])
    aea_full = np.concatenate([np.abs(ea), np.ones(N, np.float32)]).astype(np.float32)

    counts = np.bincount(row_full, minlength=N)
    counts_pad = np.concatenate([counts, np.zeros(NPAD - N, np.int64)])
    order_nodes = np.argsort(-counts_pad, kind="stable")
    # snake-assign sorted nodes to (device, local_pos) to balance edge counts
    g = np.arange(NPAD) // NDEV
    r = np.arange(NPAD) % NDEV
    dev_of_sorted = np.where(g % 2 == 0, r, NDEV - 1 - r).astype(np.int32)
    loc_of_sorted = g.astype(np.int32)
    node_dev = np.empty(NPAD, np.int32)
    node_loc = np.empty(NPAD, np.int32)
    node_dev[order_nodes] = dev_of_sorted
    node_loc[order_nodes] = loc_of_sorted
    node_glb = node_dev.astype(np.int64) * SEG_PER_DEV + node_loc

    # shared per-tile K schedule (max count in tile across all devices)
    counts_sorted = counts_pad[order_nodes]
    tile_of_sorted = loc_of_sorted // P
    Ksched = np.zeros(NTILES, np.int64)
    np.maximum.at(Ksched, tile_of_sorted, counts_sorted)
    Ksched = np.maximum(Ksched, 2)
    Ksched = ((Ksched + 1) // 2) * 2

    # group layout: per group gi, tiles j = gi*GROUP..+GROUP
    Kg = Ksched.reshape(NGROUPS, GROUP)
    SK = Kg.sum(axis=1)                      # slots per partition-row per group
    off_in_group = np.concatenate(
        [np.zeros((NGROUPS, 1), np.int64), np.cumsum(Kg, axis=1)[:, :-1]], axis=1
    )
    group_base = np.concatenate([[0], np.cumsum(P * SK)])  # slot base per group
    TOT = int(group_base[-1])

    # per-edge placement
    e_dev = node_dev[row_full]
    e_loc = node_loc[row_full]
    sort_key = e_dev.astype(np.int64) * NPAD + e_loc
    order_e = np.argsort(sort_key, kind="stable")
    sk = sort_key[order_e]
    starts = np.flatnonzero(np.concatenate([[True], sk[1:] != sk[:-1]]))
    run_id = np.cumsum(np.concatenate([[False], sk[1:] != sk[:-1]]))
    kpos = np.empty(M, np.int64)
    kpos[order_e] = np.arange(M) - starts[run_id]

    tile_j = (e_loc // P).astype(np.int64)
    part_p = (e_loc % P).astype(np.int64)
    gi = tile_j // GROUP
    jj = tile_j % GROUP
    pos = group_base[gi] + part_p * SK[gi] + off_in_group[gi, jj] + kpos

    colg = np.empty((NDEV, TOT), np.int32)
    colg[:] = (np.arange(NDEV, dtype=np.int32) * SEG_PER_DEV + SENTINEL_LOCAL)[:, None]
    aead = np.ones((NDEV, TOT), np.float32)
    colg[e_dev, pos] = node_glb[col_full].astype(np.int32)
    aead[e_dev, pos] = aea_full

    # meta arrays: per group block [P, 2*SK]: [ci row (SK) | ae row (SK)]
    meta = np.empty((NDEV, 2 * TOT), np.int32)
    for gidx in range(NGROUPS):
        b0 = int(group_base[gidx])
        skg = int(SK[gidx])
        blk_ci = colg[:, b0:b0 + P * skg].reshape(NDEV, P, skg)
        blk_ae = aead[:, b0:b0 + P * skg].reshape(NDEV, P, skg).view(np.int32)
        m2 = np.concatenate([blk_ci, blk_ae], axis=2)  # [NDEV, P, 2*SK]
        meta[:, 2 * b0:2 * (b0 + P * skg)] = m2.reshape(NDEV, -1)

    return dict(
        row_full=row_full, col_full=col_full,
        node_glb=node_glb, Ksched=Ksched, SK=SK,
        off_in_group=off_in_group, group_base=group_base, TOT=TOT,
        e_dev=e_dev, pos=pos, meta=meta,
    )


# --------------------------------------------------------------------------
# Device kernel
# --------------------------------------------------------------------------

def _build_bass(Ksched, SK, off_in_group, group_base, TOT, debug_dump=None):
    import concourse.bass as bass
    import concourse.mybir as mybir
    import concourse.tile as tile
    from concourse import bacc
    from concourse.bass import IndirectOffsetOnAxis
    from concourse.ap import AP

    f32 = mybir.dt.float32
    i32 = mybir.dt.int32
    Alu = mybir.AluOpType
    Act = mybir.ActivationFunctionType
    X = mybir.AxisListType.X

    nc = bacc.Bacc("TRN2", target_bir_lowering=False, debug=False,
                   num_devices=NDEV, dynamic_dma_scratch_size=1 << 16)

    xt = nc.dram_tensor("xt", [P, SEG_PER_DEV], f32, kind="ExternalInput")
    wb = nc.dram_tensor("wb", [P, 2 * H], f32, kind="ExternalInput")
    b2 = nc.dram_tensor("b2", [1, 2 * H], f32, kind="ExternalInput")
    meta = nc.dram_tensor("meta", [2 * TOT], i32, kind="ExternalInput")
    outd = nc.dram_tensor("out", [TOT, H], f32, kind="ExternalOutput")

    def bc(ap, ins_at, n):
        """Insert a broadcast (step-0) dim of size n at position ins_at."""
        dims = list(ap.ap)
        dims.insert(ins_at, [0, n])
        return AP(ap.tensor, ap.offset, dims)

    with tile.TileContext(nc) as tc:
        with (
            tc.tile_pool(name="const", bufs=1) as cpool,
            tc.tile_pool(name="work", bufs=2) as wpool,
            tc.tile_pool(name="small", bufs=3) as spool,
            tc.tile_pool(name="psum", bufs=2, space="PSUM") as ppool,
            tc.tile_pool(name="dram", bufs=1, space="DRAM") as dpool,
        ):
            # ---------- phase A: u, v = x @ [W1|W2] + [b|0] ----------
            xt_sb = cpool.tile([P, SEG_PER_DEV], f32)
            nc.sync.dma_start(out=xt_sb[:, :], in_=xt.ap())
            wb_sb = cpool.tile([P, 2 * H], f32)
            nc.sync.dma_start(out=wb_sb[:, :], in_=wb.ap())
            b2_sb = cpool.tile([1, 2 * H], f32)
            nc.sync.dma_start(out=b2_sb[:, :], in_=b2.ap())
            ones_sb = cpool.tile([1, P], f32)
            nc.vector.memset(ones_sb[:, :], 1.0)

            u_sb = cpool.tile([P, NTILES * H], f32)
            v_st = cpool.tile([P, NTILES * H], f32)
            for j in range(NTILES):
                ps = ppool.tile([P, 2 * H], f32, tag="ps")
                nc.tensor.matmul(ps[:, :], lhsT=xt_sb[:, j * P:(j + 1) * P],
                                 rhs=wb_sb[:, :], start=True, stop=False)
                nc.tensor.matmul(ps[:, :], lhsT=ones_sb[:, :], rhs=b2_sb[:, :],
                                 start=False, stop=True)
                nc.scalar.copy(u_sb[:, j * H:(j + 1) * H], ps[:, 0:H])
                nc.vector.tensor_copy(v_st[:, j * H:(j + 1) * H], ps[:, H:2 * H])
            # (sentinel row: host plants an x row with x@W2 = SENTINEL_VAL)
            v_shard = dpool.tile([SEG_PER_DEV, H], f32)
            v_all = dpool.tile([NPAD, H], f32, addr_space="Shared")
            nc.sync.dma_start(
                out=v_shard[:, :].rearrange("(j p) h -> p j h", p=P),
                in_=v_st[:, :].rearrange("p (j h) -> p j h", h=H),
            )
            nc.gpsimd.collective_compute(
                "AllGather", mybir.AluOpType.bypass,
                replica_groups=[list(range(NDEV))],
                ins=[v_shard[:, :].opt()],
                outs=[v_all[:, :].opt()],
            )

            # ---------- phase B: per-group gather + per-tile softmax ----------
            out_flat = outd.ap().rearrange("t h -> (t h)")
            meta_flat = meta.ap()
            for gi in range(NGROUPS):
                skg = int(SK[gi])
                gb = int(group_base[gi])
                mg = wpool.tile([P, 2 * skg], i32, tag="mg")
                nc.sync.dma_start(
                    out=mg[:, :],
                    in_=meta_flat[2 * gb:2 * (gb + P * skg)].rearrange(
                        "(p x) -> p x", x=2 * skg),
                )
                vg = wpool.tile([P, skg * H], f32, tag="vg")
                # gather v rows (bypass; no dep on prefill) one call per column
                for kk in range(skg):
                    nc.gpsimd.indirect_dma_start(
                        out=vg[:, kk * H:(kk + 1) * H],
                        out_offset=None,
                        in_=v_all[:, :],
                        in_offset=IndirectOffsetOnAxis(ap=mg[:, kk:kk + 1], axis=0),
                    )
                gg = wpool.tile([P, skg * H], f32, tag="gg")
                for jj in range(GROUP):
                    j = gi * GROUP + jj
                    K = int(Ksched[j])
                    off = int(off_in_group[gi][jj])
                    ub = u_sb[:, j * H:(j + 1) * H]
                    nc.vector.tensor_tensor(
                        out=gg[:, off * H:(off + K) * H].rearrange(
                            "p (k h) -> p k h", h=H),
                        in0=vg[:, off * H:(off + K) * H].rearrange(
                            "p (k h) -> p k h", h=H),
                        in1=bc(ub, 1, K),
                        op=Alu.add,
                    )
                # t = g * |ea| ; la = leaky(t) = max(t, 0.2t)
                ae_ap = mg[:, skg:2 * skg].bitcast(f32)
                tg = wpool.tile([P, skg * H], f32, tag="tg")
                nc.vector.tensor_tensor(
                    out=tg[:, :].rearrange("p (k h) -> p k h", h=H),
                    in0=gg[:, :].rearrange("p (k h) -> p k h", h=H),
                    in1=bc(ae_ap, 2, H),
                    op=Alu.mult,
                )
                la = wpool.tile([P, skg * H], f32, tag="la")
                nc.vector.scalar_tensor_tensor(
                    out=la[:, :], in0=tg[:, :], scalar=0.2, in1=tg[:, :],
                    op0=Alu.mult, op1=Alu.max,
                )
                mx = spool.tile([P, GROUP * H], f32, tag="mx")
                sm = spool.tile([P, GROUP * H], f32, tag="sm")
                rc = spool.tile([P, GROUP * H], f32, tag="rc")
                eg = wpool.tile([P, skg * H], f32, tag="eg")
                og = wpool.tile([P, skg * H], f32, tag="og")
                for jj in range(GROUP):
                    j = gi * GROUP + jj
                    K = int(Ksched[j])
                    off = int(off_in_group[gi][jj])
                    sl = slice(off * H, (off + K) * H)
                    la_t = la[:, sl]
                    mx_t = mx[:, jj * H:(jj + 1) * H]
                    nc.vector.tensor_reduce(
                        out=mx_t, in_=la_t.rearrange("p (k h) -> p h k", h=H),
                        axis=X, op=Alu.max,
                    )
                    # d = la - m  (into eg slice temp), e = exp(100 d)
                    nc.vector.tensor_tensor(
                        out=eg[:, sl].rearrange("p (k h) -> p k h", h=H),
                        in0=la_t.rearrange("p (k h) -> p k h", h=H),
                        in1=bc(mx_t, 1, K),
                        op=Alu.subtract,
                    )
                    nc.scalar.activation(
                        out=eg[:, sl], in_=eg[:, sl], func=Act.Exp, scale=100.0,
                    )
                    nc.vector.tensor_reduce(
                        out=sm[:, jj * H:(jj + 1) * H],
                        in_=eg[:, sl].rearrange("p (k h) -> p h k", h=H),
                        axis=X, op=Alu.add,
                    )
                nc.vector.reciprocal(rc[:, :], sm[:, :])
                for jj in range(GROUP):
                    j = gi * GROUP + jj
                    K = int(Ksched[j])
                    off = int(off_in_group[gi][jj])
                    sl = slice(off * H, (off + K) * H)
                    nc.vector.tensor_tensor(
                        out=og[:, sl].rearrange("p (k h) -> p k h", h=H),
                        in0=eg[:, sl].rearrange("p (k h) -> p k h", h=H),
                        in1=bc(rc[:, jj * H:(jj + 1) * H], 1, K),
                        op=Alu.mult,
                    )
                dump = {"g": gg, "t": tg, "la": la, "e": eg, None: og}[debug_dump]
                nc.sync.dma_start(
                    out=out_flat[gb * H:(gb + P * skg) * H].rearrange(
                        "(p x) -> p x", x=skg * H),
                    in_=dump[:, :],
                )
    nc.compile()
    return nc


# --------------------------------------------------------------------------
# Entry point
# --------------------------------------------------------------------------

def _prepare(inputs):
    plan = _build_plan(inputs["edge_index"], inputs["edge_attr"])
    x = np.asarray(inputs["x"], np.float32)
    W = np.asarray(inputs["W"], np.float32)
    b = np.asarray(inputs["b"], np.float32)

    x_glb = np.zeros((NPAD, C), np.float32)
    x_glb[plan["node_glb"][:N]] = x
    # sentinel: craft an x row so x @ W2 == SENTINEL_VAL on every head
    # (min-norm solve; the sentinel position is a dummy node on each device)
    W2 = W[C:].astype(np.float64)
    xs = np.linalg.lstsq(W2.T, np.full(H, SENTINEL_VAL), rcond=None)[0]
    for d in range(NDEV):
        x_glb[d * SEG_PER_DEV + SENTINEL_LOCAL] = xs.astype(np.float32)
    xt_full = np.ascontiguousarray(x_glb.T)  # [C, NPAD]
    wbm = np.ascontiguousarray(np.concatenate([W[:C], W[C:]], axis=1))  # [128,16]
    b2 = np.concatenate([b, np.zeros(H, np.float32)])[None, :].astype(np.float32)

    in_maps = []
    for d in range(NDEV):
        in_maps.append({
            "xt": np.ascontiguousarray(
                xt_full[:, d * SEG_PER_DEV:(d + 1) * SEG_PER_DEV]),
            "wb": wbm,
            "b2": b2,
            "meta": np.ascontiguousarray(plan["meta"][d]),
        })
    return plan, in_maps


def _get_nc(plan, debug_dump=None):
    key = (tuple(plan["Ksched"].tolist()), debug_dump)
    if key not in _COMPILED:
        _COMPILED[key] = _build_bass(
            plan["Ksched"], plan["SK"], plan["off_in_group"],
            plan["group_base"], plan["TOT"], debug_dump=debug_dump)
    return _COMPILED[key]


def run(inputs, trace=False, debug_dump=None):
    """Returns ((alpha, rowcol), BassKernelResults)."""
    from concourse.bass_utils import run_bass_kernel_spmd

    plan, in_maps = _prepare(inputs)
    nc = _get_nc(plan, debug_dump)
    res = run_bass_kernel_spmd(nc, in_maps, core_ids=list(range(NDEV)),
                               trace=trace)
    outs = np.stack([r["out"] for r in res.results])  # [NDEV, TOT, H]
    alpha = outs[plan["e_dev"], plan["pos"]].astype(np.float32)
    rowcol = np.stack([plan["row_full"], plan["col_full"]]).astype(np.int32)
    return (alpha, rowcol), res


def kernel(**inputs):
    (alpha, rowcol), _ = run(inputs, trace=False)
    return alpha, rowcol


if __name__ == "__main__":
    from host_proto import np_setup_inputs, np_reference
    ins = np_setup_inputs()
    (alpha, rowcol), res = run(ins, trace=False)
    exp, rc = np_reference(**ins)
    err = np.linalg.norm(alpha - exp) / np.linalg.norm(exp)
    print("rel l2:", err)
    print("max abs diff:", np.abs(alpha - exp).max())
    print("rowcol ok:", np.array_equal(rowcol, rc))


# revision 15
# speedup vs baseline: 1.5007x; 1.0385x over previous
"""GNN edge-softmax attention kernel for 8 TRN2 NeuronCores.

Problem: nn_Attention_3015067042351 (gnn_message_passing).
  N=50000 nodes, E=1600000 edges, C=128, H=8.
  alpha = segment_softmax(leaky_relu((x[row]@W1 + x[col]@W2 + b)*|ea|)*100, by=row)

Strategy:
  - Algebra: x[row]@W1 + x[col]@W2 = u[row] + v[col] with u = x@W1+b, v = x@W2
    (per-edge gather shrinks from 512B to 32B).
  - Host "sharding": partition nodes (softmax segments) across 8 devices by
    edge count (snake order) so per-device load balances; sort segments by
    count so tiles of 128 segments share a small per-tile K (max edges/seg);
    lay edges out dense [segment, K] so the softmax is tile-local (no
    cross-device reduction, no scatter).
  - Device: matmul u,v for the local node shard; AllGather v; per tile of
    128 segments gather v[col] rows via indirect DMA (accumulated onto a
    broadcast u prefill), then mul/leaky/max/exp/sum/normalize; write out.
  - Host unpermutes dense output back to original edge order.

Self-contained: hardcodes shapes; only needs the /opt/trn_rl_repo runtime.
"""

import sys

if "/opt/trn_rl_repo" not in sys.path:
    sys.path.insert(0, "/opt/trn_rl_repo")

import numpy as np

N, E, C, H = 50000, 1600000, 128, 8
NDEV = 8
P = 128
NTILES = 49            # tiles of 128 segments per device
SEG_PER_DEV = NTILES * P   # 6272
NPAD = NDEV * SEG_PER_DEV  # 50176
SENTINEL_LOCAL = SEG_PER_DEV - 1  # per-device v row holding -1e33 (masks pads)
GROUP = 1              # tiles per DMA/gather group
NGROUPS = NTILES // GROUP
SENTINEL_VAL = -1.0e33

_COMPILED = {}  # (Ksched tuple) -> (nc, names)


# --------------------------------------------------------------------------
# Host-side plan
# --------------------------------------------------------------------------

def _build_plan(edge_index, edge_attr):
    row = np.asarray(edge_index[0])
    col = np.asarray(edge_index[1])
    ea = np.asarray(edge_attr, dtype=np.float32)
    M = E + N
    # reference-order edge list (edges then loops) for outputs
    row_full = np.concatenate([row, np.arange(N, dtype=row.dtype)])
    col_full = np.concatenate([col, np.arange(N, dtype=col.dtype)])
    # layout-order edge list: LOOPS FIRST so each segment's k=0 slot is its
    # self-loop (v[own segment] is SBUF-local -> no gather for column 0)
    row2 = np.concatenate([np.arange(N, dtype=row.dtype), row])
    col2 = np.concatenate([np.arange(N, dtype=col.dtype), col])
    aea2 = np.concatenate([np.ones(N, np.float32), np.abs(ea)]).astype(np.float32)
    perm_of_ref = np.concatenate([np.arange(N, N + E), np.arange(N)])

    counts = np.bincount(row_full, minlength=N)
    counts_pad = np.concatenate([counts, np.zeros(NPAD - N, np.int64)])
    order_nodes = np.argsort(-counts_pad, kind="stable")
    # snake-assign sorted nodes to (device, local_pos) to balance edge counts
    g = np.arange(NPAD) // NDEV
    r = np.arange(NPAD) % NDEV
    dev_of_sorted = np.where(g % 2 == 0, r, NDEV - 1 - r).astype(np.int32)
    loc_of_sorted = g.astype(np.int32)
    node_dev = np.empty(NPAD, np.int32)
    node_loc = np.empty(NPAD, np.int32)
    node_dev[order_nodes] = dev_of_sorted
    node_loc[order_nodes] = loc_of_sorted
    node_glb = node_dev.astype(np.int64) * SEG_PER_DEV + node_loc

    # shared per-tile K schedule (max count in tile across all devices)
    counts_sorted = counts_pad[order_nodes]
    tile_of_sorted = loc_of_sorted // P
    Ksched = np.zeros(NTILES, np.int64)
    np.maximum.at(Ksched, tile_of_sorted, counts_sorted)
    Ksched = np.maximum(Ksched, 2)
    Ksched = ((Ksched + 1) // 2) * 2

    # group layout: per group gi, tiles j = gi*GROUP..+GROUP
    Kg = Ksched.reshape(NGROUPS, GROUP)
    SK = Kg.sum(axis=1)                      # slots per partition-row per group
    off_in_group = np.concatenate(
        [np.zeros((NGROUPS, 1), np.int64), np.cumsum(Kg, axis=1)[:, :-1]], axis=1
    )
    group_base = np.concatenate([[0], np.cumsum(P * SK)])  # slot base per group
    TOT = int(group_base[-1])

    # per-edge placement (layout order: loops first)
    e_dev = node_dev[row2]
    e_loc = node_loc[row2]
    sort_key = e_dev.astype(np.int64) * NPAD + e_loc
    order_e = np.argsort(sort_key, kind="stable")
    sk = sort_key[order_e]
    starts = np.flatnonzero(np.concatenate([[True], sk[1:] != sk[:-1]]))
    run_id = np.cumsum(np.concatenate([[False], sk[1:] != sk[:-1]]))
    kpos = np.empty(M, np.int64)
    kpos[order_e] = np.arange(M) - starts[run_id]

    tile_j = (e_loc // P).astype(np.int64)
    part_p = (e_loc % P).astype(np.int64)
    gi = tile_j // GROUP
    jj = tile_j % GROUP
    pos = group_base[gi] + part_p * SK[gi] + off_in_group[gi, jj] + kpos

    colg = np.empty((NDEV, TOT), np.int32)
    colg[:] = (np.arange(NDEV, dtype=np.int32) * SEG_PER_DEV + SENTINEL_LOCAL)[:, None]
    aead = np.ones((NDEV, TOT), np.float32)
    colg[e_dev, pos] = node_glb[col2].astype(np.int32)
    aead[e_dev, pos] = aea2

    # meta arrays: per group block [P, 2*SK]: [ci row (SK) | ae row (SK)]
    meta = np.empty((NDEV, 2 * TOT), np.int32)
    for gidx in range(NGROUPS):
        b0 = int(group_base[gidx])
        skg = int(SK[gidx])
        blk_ci = colg[:, b0:b0 + P * skg].reshape(NDEV, P, skg)
        blk_ae = aead[:, b0:b0 + P * skg].reshape(NDEV, P, skg).view(np.int32)
        m2 = np.concatenate([blk_ci, blk_ae], axis=2)  # [NDEV, P, 2*SK]
        meta[:, 2 * b0:2 * (b0 + P * skg)] = m2.reshape(NDEV, -1)

    return dict(
        row_full=row_full, col_full=col_full,
        node_glb=node_glb, Ksched=Ksched, SK=SK,
        off_in_group=off_in_group, group_base=group_base, TOT=TOT,
        e_dev=e_dev[perm_of_ref], pos=pos[perm_of_ref], meta=meta,
    )


# --------------------------------------------------------------------------
# Device kernel
# --------------------------------------------------------------------------

def _build_bass(Ksched, SK, off_in_group, group_base, TOT, debug_dump=None):
    import concourse.bass as bass
    import concourse.mybir as mybir
    import concourse.tile as tile
    from concourse import bacc
    from concourse.bass import IndirectOffsetOnAxis
    from concourse.ap import AP

    f32 = mybir.dt.float32
    i32 = mybir.dt.int32
    Alu = mybir.AluOpType
    Act = mybir.ActivationFunctionType
    X = mybir.AxisListType.X

    nc = bacc.Bacc("TRN2", target_bir_lowering=False, debug=False,
                   num_devices=NDEV, dynamic_dma_scratch_size=1 << 16)

    xt = nc.dram_tensor("xt", [P, SEG_PER_DEV], f32, kind="ExternalInput")
    wb = nc.dram_tensor("wb", [P, 2 * H], f32, kind="ExternalInput")
    b2 = nc.dram_tensor("b2", [1, 2 * H], f32, kind="ExternalInput")
    meta = nc.dram_tensor("meta", [2 * TOT], i32, kind="ExternalInput")
    outd = nc.dram_tensor("out", [TOT, H], f32, kind="ExternalOutput")

    def bc(ap, ins_at, n):
        """Insert a broadcast (step-0) dim of size n at position ins_at."""
        dims = list(ap.ap)
        dims.insert(ins_at, [0, n])
        return AP(ap.tensor, ap.offset, dims)

    with tile.TileContext(nc) as tc:
        with (
            tc.tile_pool(name="const", bufs=1) as cpool,
            tc.tile_pool(name="work", bufs=2) as wpool,
            tc.tile_pool(name="small", bufs=3) as spool,
            tc.tile_pool(name="psum", bufs=2, space="PSUM") as ppool,
            tc.tile_pool(name="dram", bufs=1, space="DRAM") as dpool,
        ):
            # ---------- phase A: u, v = x @ [W1|W2] + [b|0] ----------
            xt_sb = cpool.tile([P, SEG_PER_DEV], f32)
            nc.sync.dma_start(out=xt_sb[:, :], in_=xt.ap())
            wb_sb = cpool.tile([P, 2 * H], f32)
            nc.sync.dma_start(out=wb_sb[:, :], in_=wb.ap())
            b2_sb = cpool.tile([1, 2 * H], f32)
            nc.sync.dma_start(out=b2_sb[:, :], in_=b2.ap())
            ones_sb = cpool.tile([1, P], f32)
            nc.vector.memset(ones_sb[:, :], 1.0)

            u_sb = cpool.tile([P, NTILES * H], f32)
            v_st = cpool.tile([P, NTILES * H], f32)
            for j in range(NTILES):
                ps = ppool.tile([P, 2 * H], f32, tag="ps")
                nc.tensor.matmul(ps[:, :], lhsT=xt_sb[:, j * P:(j + 1) * P],
                                 rhs=wb_sb[:, :], start=True, stop=False)
                nc.tensor.matmul(ps[:, :], lhsT=ones_sb[:, :], rhs=b2_sb[:, :],
                                 start=False, stop=True)
                nc.scalar.copy(u_sb[:, j * H:(j + 1) * H], ps[:, 0:H])
                nc.vector.tensor_copy(v_st[:, j * H:(j + 1) * H], ps[:, H:2 * H])
            # (sentinel row: host plants an x row with x@W2 = SENTINEL_VAL)
            v_shard = dpool.tile([SEG_PER_DEV, H], f32)
            v_all = dpool.tile([NPAD, H], f32, addr_space="Shared")
            nc.sync.dma_start(
                out=v_shard[:, :].rearrange("(j p) h -> p j h", p=P),
                in_=v_st[:, :].rearrange("p (j h) -> p j h", h=H),
            )
            nc.gpsimd.collective_compute(
                "AllGather", mybir.AluOpType.bypass,
                replica_groups=[list(range(NDEV))],
                ins=[v_shard[:, :].opt()],
                outs=[v_all[:, :].opt()],
            )

            # ---------- phase B: per-group gather + per-tile softmax ----------
            out_flat = outd.ap().rearrange("t h -> (t h)")
            meta_flat = meta.ap()
            for gi in range(NGROUPS):
                skg = int(SK[gi])
                gb = int(group_base[gi])
                mg = wpool.tile([P, 2 * skg], i32, tag="mg")
                nc.sync.dma_start(
                    out=mg[:, :],
                    in_=meta_flat[2 * gb:2 * (gb + P * skg)].rearrange(
                        "(p x) -> p x", x=2 * skg),
                )
                vg = wpool.tile([P, skg * H], f32, tag="vg")
                # column 0 is the self-loop -> v comes from local v_st below;
                # gather the rest, one indirect call per column
                for kk in range(1, skg):
                    nc.gpsimd.indirect_dma_start(
                        out=vg[:, kk * H:(kk + 1) * H],
                        out_offset=None,
                        in_=v_all[:, :],
                        in_offset=IndirectOffsetOnAxis(ap=mg[:, kk:kk + 1], axis=0),
                    )
                gg = wpool.tile([P, skg * H], f32, tag="gg")
                for jj in range(GROUP):
                    j = gi * GROUP + jj
                    K = int(Ksched[j])
                    off = int(off_in_group[gi][jj])
                    ub = u_sb[:, j * H:(j + 1) * H]
                    # self-loop column: u + v of the OWN segment (both local)
                    nc.vector.tensor_add(
                        out=gg[:, off * H:(off + 1) * H],
                        in0=v_st[:, j * H:(j + 1) * H],
                        in1=ub,
                    )
                    nc.vector.tensor_tensor(
                        out=gg[:, (off + 1) * H:(off + K) * H].rearrange(
                            "p (k h) -> p k h", h=H),
                        in0=vg[:, (off + 1) * H:(off + K) * H].rearrange(
                            "p (k h) -> p k h", h=H),
                        in1=bc(ub, 1, K - 1),
                        op=Alu.add,
                    )
                # t = g * |ea| ; la = leaky(t) = max(t, 0.2t)
                ae_ap = mg[:, skg:2 * skg].bitcast(f32)
                tg = wpool.tile([P, skg * H], f32, tag="tg")
                nc.vector.tensor_tensor(
                    out=tg[:, :].rearrange("p (k h) -> p k h", h=H),
                    in0=gg[:, :].rearrange("p (k h) -> p k h", h=H),
                    in1=bc(ae_ap, 2, H),
                    op=Alu.mult,
                )
                la = wpool.tile([P, skg * H], f32, tag="la")
                nc.vector.scalar_tensor_tensor(
                    out=la[:, :], in0=tg[:, :], scalar=0.2, in1=tg[:, :],
                    op0=Alu.mult, op1=Alu.max,
                )
                mx = spool.tile([P, GROUP * H], f32, tag="mx")
                sm = spool.tile([P, GROUP * H], f32, tag="sm")
                rc = spool.tile([P, GROUP * H], f32, tag="rc")
                eg = wpool.tile([P, skg * H], f32, tag="eg")
                og = wpool.tile([P, skg * H], f32, tag="og")
                for jj in range(GROUP):
                    j = gi * GROUP + jj
                    K = int(Ksched[j])
                    off = int(off_in_group[gi][jj])
                    sl = slice(off * H, (off + K) * H)
                    la_t = la[:, sl]
                    mx_t = mx[:, jj * H:(jj + 1) * H]
                    nc.vector.tensor_reduce(
                        out=mx_t, in_=la_t.rearrange("p (k h) -> p h k", h=H),
                        axis=X, op=Alu.max,
                    )
                    # d = la - m  (into eg slice temp), e = exp(100 d)
                    nc.vector.tensor_tensor(
                        out=eg[:, sl].rearrange("p (k h) -> p k h", h=H),
                        in0=la_t.rearrange("p (k h) -> p k h", h=H),
                        in1=bc(mx_t, 1, K),
                        op=Alu.subtract,
                    )
                    nc.scalar.activation(
                        out=eg[:, sl], in_=eg[:, sl], func=Act.Exp, scale=100.0,
                    )
                    nc.vector.tensor_reduce(
                        out=sm[:, jj * H:(jj + 1) * H],
                        in_=eg[:, sl].rearrange("p (k h) -> p h k", h=H),
                        axis=X, op=Alu.add,
                    )
                nc.vector.reciprocal(rc[:, :], sm[:, :])
                for jj in range(GROUP):
                    j = gi * GROUP + jj
                    K = int(Ksched[j])
                    off = int(off_in_group[gi][jj])
                    sl = slice(off * H, (off + K) * H)
                    nc.vector.tensor_tensor(
                        out=og[:, sl].rearrange("p (k h) -> p k h", h=H),
                        in0=eg[:, sl].rearrange("p (k h) -> p k h", h=H),
                        in1=bc(rc[:, jj * H:(jj + 1) * H], 1, K),
                        op=Alu.mult,
                    )
                dump = {"g": gg, "t": tg, "la": la, "e": eg, None: og}[debug_dump]
                nc.sync.dma_start(
                    out=out_flat[gb * H:(gb + P * skg) * H].rearrange(
                        "(p x) -> p x", x=skg * H),
                    in_=dump[:, :],
                )
    nc.compile()
    return nc


# --------------------------------------------------------------------------
# Entry point
# --------------------------------------------------------------------------

def _prepare(inputs):
    plan = _build_plan(inputs["edge_index"], inputs["edge_attr"])
    x = np.asarray(inputs["x"], np.float32)
    W = np.asarray(inputs["W"], np.float32)
    b = np.asarray(inputs["b"], np.float32)

    x_glb = np.zeros((NPAD, C), np.float32)
    x_glb[plan["node_glb"][:N]] = x
    # sentinel: craft an x row so x @ W2 == SENTINEL_VAL on every head
    # (min-norm solve; the sentinel position is a dummy node on each device)
    W2 = W[C:].astype(np.float64)
    xs = np.linalg.lstsq(W2.T, np.full(H, SENTINEL_VAL), rcond=None)[0]
    for d in range(NDEV):
        x_glb[d * SEG_PER_DEV + SENTINEL_LOCAL] = xs.astype(np.float32)
    xt_full = np.ascontiguousarray(x_glb.T)  # [C, NPAD]
    wbm = np.ascontiguousarray(np.concatenate([W[:C], W[C:]], axis=1))  # [128,16]
    b2 = np.concatenate([b, np.zeros(H, np.float32)])[None, :].astype(np.float32)

    in_maps = []
    for d in range(NDEV):
        in_maps.append({
            "xt": np.ascontiguousarray(
                xt_full[:, d * SEG_PER_DEV:(d + 1) * SEG_PER_DEV]),
            "wb": wbm,
            "b2": b2,
            "meta": np.ascontiguousarray(plan["meta"][d]),
        })
    return plan, in_maps


def _get_nc(plan, debug_dump=None):
    key = (tuple(plan["Ksched"].tolist()), debug_dump)
    if key not in _COMPILED:
        _COMPILED[key] = _build_bass(
            plan["Ksched"], plan["SK"], plan["off_in_group"],
            plan["group_base"], plan["TOT"], debug_dump=debug_dump)
    return _COMPILED[key]


def run(inputs, trace=False, debug_dump=None):
    """Returns ((alpha, rowcol), BassKernelResults)."""
    from concourse.bass_utils import run_bass_kernel_spmd

    plan, in_maps = _prepare(inputs)
    nc = _get_nc(plan, debug_dump)
    res = run_bass_kernel_spmd(nc, in_maps, core_ids=list(range(NDEV)),
                               trace=trace)
    outs = np.stack([r["out"] for r in res.results])  # [NDEV, TOT, H]
    alpha = outs[plan["e_dev"], plan["pos"]].astype(np.float32)
    rowcol = np.stack([plan["row_full"], plan["col_full"]]).astype(np.int32)
    return (alpha, rowcol), res


def kernel(**inputs):
    (alpha, rowcol), _ = run(inputs, trace=False)
    return alpha, rowcol


if __name__ == "__main__":
    from host_proto import np_setup_inputs, np_reference
    ins = np_setup_inputs()
    (alpha, rowcol), res = run(ins, trace=False)
    exp, rc = np_reference(**ins)
    err = np.linalg.norm(alpha - exp) / np.linalg.norm(exp)
    print("rel l2:", err)
    print("max abs diff:", np.abs(alpha - exp).max())
    print("rowcol ok:", np.array_equal(rowcol, rc))
